# revision 1
# baseline (speedup 1.0000x reference)
"""GPT-2 decode-step (attention w/ KV cache + MLP) on 8 Trainium2 cores.

Sharding: tensor-parallel over heads (2 heads/core) for attention,
and over the 8192 intermediate dim (1024/core) for the MLP.
Two SPMD launches with a tiny host reduction between (LN2 needs full h).
"""

import sys

for _p in ("/opt/trn_rl_repo",):
    if _p not in sys.path:
        sys.path.append(_p)

import numpy as np

import concourse.bass as bass
import concourse.bacc as bacc
import concourse.mybir as mybir
from concourse import tile
from concourse.bass_utils import run_bass_kernel_spmd


def _hw_nc():
    return bacc.Bacc("TRN2", target_bir_lowering=False, debug=False)

FP = mybir.dt.float32
P = 128
EPS = 1e-5
AF = mybir.ActivationFunctionType


# ---------------------------------------------------------------------------
# Phase 1: LN1 + qkv (local heads) + attention over KV cache + proj partial
# ---------------------------------------------------------------------------
def build_phase1(B=16, S=4096, H=2048, HD=128, NHL=2, nc_factory=bass.Bass):
    assert HD == P
    T = S // P          # number of 128-row S tiles per (b, h)
    HC = H // P         # hidden-dim chunks
    NG = 3 * NHL        # qkv column groups of width 128: [q0..q_{NHL-1} k.. v..]
    NJ = NHL * B        # number of (h, b) attention problems on this core
    s_scale = 1.0 / float(np.sqrt(HD))

    nc = nc_factory()
    hid = nc.declare_dram_parameter("hid", [B, H], FP, isOutput=False)
    ln1g = nc.declare_dram_parameter("ln1g", [B, H], FP, isOutput=False)
    ln1b = nc.declare_dram_parameter("ln1b", [B, H], FP, isOutput=False)
    wqkv = nc.declare_dram_parameter("wqkv", [H, NG * P], FP, isOutput=False)
    bqkv = nc.declare_dram_parameter("bqkv", [NG * P], FP, isOutput=False)
    kc = nc.declare_dram_parameter("kc", [B, NHL, S, HD], FP, isOutput=False)
    vc = nc.declare_dram_parameter("vc", [B, NHL, S, HD], FP, isOutput=False)
    wproj = nc.declare_dram_parameter("wproj", [NHL * HD, H], FP, isOutput=False)
    ident = nc.declare_dram_parameter("ident", [P, P], FP, isOutput=False)
    identB = nc.declare_dram_parameter("identB", [B, B], FP, isOutput=False)
    onesc = nc.declare_dram_parameter("onesc", [P, 1], FP, isOutput=False)
    onesr = nc.declare_dram_parameter("onesr", [1, P], FP, isOutput=False)
    hpart = nc.declare_dram_parameter("hpart", [B, H], FP, isOutput=True)

    with tile.TileContext(nc) as tc:
        with (
            tc.tile_pool(name="const", bufs=1) as constp,
            tc.tile_pool(name="pers", bufs=1) as pers,
            tc.tile_pool(name="persL", bufs=1, space="PSUM") as persL,
        ):
            id_sb = constp.tile([P, P], FP)
            nc.sync.dma_start(id_sb[:], ident[:])
            idB_sb = constp.tile([B, B], FP)
            nc.sync.dma_start(idB_sb[:], identB[:])
            ones_sb = constp.tile([P, 1], FP)
            nc.sync.dma_start(ones_sb[:], onesc[:])
            onesr_sb = constp.tile([1, P], FP)
            nc.sync.dma_start(onesr_sb[:], onesr[:])
            bq_sb = constp.tile([P, NG], FP)
            nc.sync.dma_start(bq_sb[:], bqkv.rearrange("(g p) -> p g", p=P))

            # persistent across the attention loop
            qkvT_sb = pers.tile([P, NG * B], FP)     # [HD, (g, b)]
            O_sb = pers.tile([P, NJ], FP)            # unnormalized attn out
            L_sb = pers.tile([1, NJ], FP)            # softmax denominators
            wproj_sb = pers.tile([P, NHL, H], FP)    # W_proj rows (per head)
            nc.sync.dma_start(wproj_sb[:], wproj.rearrange("(h p) c -> p h c", p=P))
            psum_L = persL.tile([1, NJ], FP)

            # ---------------- preamble: LN1 + qkvT ----------------
            with (
                tc.tile_pool(name="pre", bufs=1) as pre,
                tc.tile_pool(name="prew", bufs=1) as prew,
                tc.tile_pool(name="prep", bufs=2, space="PSUM") as prep,
            ):
                hid_sb = pre.tile([B, H], FP)
                nc.sync.dma_start(hid_sb[:], hid[:])
                g_sb = pre.tile([B, H], FP)
                nc.sync.dma_start(g_sb[:], ln1g[:])
                b_sb = pre.tile([B, H], FP)
                nc.sync.dma_start(b_sb[:], ln1b[:])
                wqkv_sb = prew.tile([P, HC, NG, P], FP)
                nc.sync.dma_start(
                    wqkv_sb[:], wqkv.rearrange("(hc p) (g f) -> p hc g f", p=P, g=NG)
                )

                mu = pre.tile([B, 1], FP)
                nc.vector.reduce_sum(mu[:], hid_sb[:], axis=mybir.AxisListType.X)
                nc.scalar.mul(mu[:], mu[:], 1.0 / H)
                xc = pre.tile([B, H], FP)
                nc.vector.tensor_scalar_sub(xc[:], hid_sb[:], mu[:, 0:1])
                sq = pre.tile([B, H], FP)
                nc.vector.tensor_mul(sq[:], xc[:], xc[:])
                vsum = pre.tile([B, 1], FP)
                nc.vector.reduce_sum(vsum[:], sq[:], axis=mybir.AxisListType.X)
                eps_t = pre.tile([B, 1], FP)
                nc.vector.memset(eps_t[:], EPS)
                stddev = pre.tile([B, 1], FP)
                nc.scalar.activation(
                    stddev[:], vsum[:], AF.Sqrt, bias=eps_t[:, 0:1], scale=1.0 / H
                )
                rstd = pre.tile([B, 1], FP)
                nc.vector.reciprocal(rstd[:], stddev[:])
                xh = pre.tile([B, H], FP)
                nc.vector.tensor_scalar_mul(xh[:], xc[:], rstd[:, 0:1])
                nc.vector.tensor_mul(xh[:], xh[:], g_sb[:])
                nc.vector.tensor_add(xh[:], xh[:], b_sb[:])

                # transpose x-hat -> xT [H-chunks on partitions, B]
                xT_sb = pre.tile([P, HC * B], FP)
                for hcc in range(HC):
                    pt = prep.tile([P, B], FP, tag="pt")
                    nc.tensor.transpose(pt[:], xh[:, hcc * P:(hcc + 1) * P], idB_sb[:])
                    nc.scalar.copy(xT_sb[:, hcc * B:(hcc + 1) * B], pt[:])

                # qkvT = W_slice.T @ xhat.T  -> [128 (col grp), B] per group
                for g in range(NG):
                    pq = prep.tile([P, B], FP, tag="pq")
                    for hcc in range(HC):
                        nc.tensor.matmul(
                            pq[:],
                            wqkv_sb[:, hcc, g, :],
                            xT_sb[:, hcc * B:(hcc + 1) * B],
                            start=(hcc == 0),
                            stop=(hcc == HC - 1),
                        )
                    # q groups are pre-scaled by 1/sqrt(HD); bias comes in
                    # pre-scaled from the host for those groups too.
                    scl = s_scale if g < NHL else 1.0
                    nc.scalar.activation(
                        qkvT_sb[:, g * B:(g + 1) * B], pq[:], AF.Identity,
                        bias=bq_sb[:, g:g + 1], scale=scl,
                    )

            # ---------------- main attention loop ----------------
            with (
                tc.tile_pool(name="kv", bufs=2) as kvp,
                tc.tile_pool(name="ktr", bufs=2) as ktp,
                tc.tile_pool(name="esb", bufs=2) as ep,
                tc.tile_pool(name="ptt", bufs=3, space="PSUM") as pst,
                tc.tile_pool(name="psc", bufs=2, space="PSUM") as pscp,
                tc.tile_pool(name="po", bufs=2, space="PSUM") as pop,
            ):
                for b in range(B):
                    kbuf = kvp.tile([P, NHL, T, P], FP, tag="kbuf")
                    nc.sync.dma_start(
                        kbuf[:], kc[b].rearrange("h (t p) d -> p h t d", p=P)
                    )
                    vbuf = kvp.tile([P, NHL, T, P], FP, tag="vbuf")
                    nc.sync.dma_start(
                        vbuf[:], vc[b].rearrange("h (t p) d -> p h t d", p=P)
                    )
                    for h in range(NHL):
                        j = h * B + b
                        ktr = ktp.tile([P, T * P], FP, tag="ktr")
                        for t in range(T):
                            ptt = pst.tile([P, P], FP, tag="ptt")
                            nc.tensor.transpose(ptt[:], kbuf[:, h, t, :], id_sb[:])
                            nc.vector.tensor_copy(ktr[:, t * P:(t + 1) * P], ptt[:])
                        psc = pscp.tile([P, T], FP, tag="psc")
                        for t in range(T):
                            nc.tensor.matmul(
                                psc[:, t:t + 1],
                                ktr[:, t * P:(t + 1) * P],
                                qkvT_sb[:, h * B + b:h * B + b + 1],
                                start=True, stop=True,
                            )
                        e_sb = ep.tile([P, T], FP, tag="e_sb")
                        esum = ep.tile([P, 1], FP, tag="esum")
                        nc.scalar.activation(
                            e_sb[:], psc[:], AF.Exp, accum_out=esum[:]
                        )
                        nc.tensor.matmul(
                            psum_L[0:1, j:j + 1], esum[:], ones_sb[:],
                            start=True, stop=True,
                        )
                        po = pop.tile([P, 1], FP, tag="po")
                        for t in range(T):
                            nc.tensor.matmul(
                                po[:], vbuf[:, h, t, :], e_sb[:, t:t + 1],
                                start=(t == 0), stop=(t == T - 1),
                            )
                        nc.scalar.copy(O_sb[:, j:j + 1], po[:])

            # ---------------- epilogue: new token + normalize + proj ----------
            with (
                tc.tile_pool(name="post", bufs=1) as post,
                tc.tile_pool(name="postp", bufs=1, space="PSUM") as postp,
            ):
                nc.vector.tensor_copy(L_sb[:], psum_L[:])
                for h in range(NHL):
                    pq = post.tile([P, B], FP, tag="pq2")
                    nc.vector.tensor_mul(
                        pq[:],
                        qkvT_sb[:, h * B:(h + 1) * B],
                        qkvT_sb[:, (NHL + h) * B:(NHL + h + 1) * B],
                    )
                    psn = postp.tile([1, B], FP, tag="psn")
                    nc.tensor.matmul(psn[:], ones_sb[:], pq[:], start=True, stop=True)
                    en = post.tile([1, B], FP, tag="en")
                    nc.scalar.activation(en[:], psn[:], AF.Exp)
                    nc.vector.tensor_add(
                        L_sb[:, h * B:(h + 1) * B], L_sb[:, h * B:(h + 1) * B], en[:]
                    )
                    pbc = postp.tile([P, B], FP, tag="pbc")
                    nc.tensor.matmul(pbc[:], onesr_sb[:], en[:], start=True, stop=True)
                    vn = post.tile([P, B], FP, tag="vn")
                    nc.vector.tensor_mul(
                        vn[:], qkvT_sb[:, (2 * NHL + h) * B:(2 * NHL + h + 1) * B],
                        pbc[:],
                    )
                    nc.vector.tensor_add(
                        O_sb[:, h * B:(h + 1) * B], O_sb[:, h * B:(h + 1) * B], vn[:]
                    )
                linv = post.tile([1, NJ], FP)
                nc.vector.reciprocal(linv[:], L_sb[:])
                plinv = postp.tile([P, NJ], FP)
                nc.tensor.matmul(plinv[:], onesr_sb[:], linv[:], start=True, stop=True)
                nc.vector.tensor_mul(O_sb[:], O_sb[:], plinv[:])

                hp_sb = post.tile([B, H], FP)
                NSPL = H // 512
                for n in range(NSPL):
                    ppr = postp.tile([B, 512], FP, tag="ppr")
                    for h in range(NHL):
                        nc.tensor.matmul(
                            ppr[:],
                            O_sb[:, h * B:(h + 1) * B],
                            wproj_sb[:, h, n * 512:(n + 1) * 512],
                            start=(h == 0), stop=(h == NHL - 1),
                        )
                    nc.scalar.copy(hp_sb[:, n * 512:(n + 1) * 512], ppr[:])
                nc.sync.dma_start(hpart[:], hp_sb[:])
    return nc


# ---------------------------------------------------------------------------
# Phase 2: MLP partial (intermediate-dim shard), input is host-computed LN2(h)
# ---------------------------------------------------------------------------
def build_phase2(B=16, H=2048, I=1024, nc_factory=bass.Bass):
    HC = H // P
    IC = I // P
    nc = nc_factory()
    xh2 = nc.declare_dram_parameter("xh2", [B, H], FP, isOutput=False)
    wfc = nc.declare_dram_parameter("wfc", [H, I], FP, isOutput=False)
    bfc = nc.declare_dram_parameter("bfc", [I], FP, isOutput=False)
    wout = nc.declare_dram_parameter("wout", [I, H], FP, isOutput=False)
    identB = nc.declare_dram_parameter("identB", [B, B], FP, isOutput=False)
    ypart = nc.declare_dram_parameter("ypart", [B, H], FP, isOutput=True)

    NW = min(512, I)   # moving width for fc (fp32 PSUM-bank limit)
    NWH = min(512, H)  # moving width for out-proj
    with tile.TileContext(nc) as tc:
        with (
            tc.tile_pool(name="sb", bufs=1) as sb,
            tc.tile_pool(name="ps", bufs=2, space="PSUM") as ps,
            tc.tile_pool(name="psu", bufs=1, space="PSUM") as psu,
        ):
            idB_sb = sb.tile([B, B], FP)
            nc.sync.dma_start(idB_sb[:], identB[:])
            xh_sb = sb.tile([B, H], FP)
            nc.sync.dma_start(xh_sb[:], xh2[:])
            bfc_sb = sb.tile([P, IC], FP)
            nc.sync.dma_start(bfc_sb[:], bfc.rearrange("(ic p) -> p ic", p=P))
            # chunked weight loads so compute starts early
            wfc_sb = sb.tile([P, HC, I], FP)
            wfc_r = wfc.rearrange("(hc p) i -> p hc i", p=P)
            nck1 = min(4, HC)
            for cc in range(nck1):
                s0, s1 = cc * HC // nck1, (cc + 1) * HC // nck1
                nc.sync.dma_start(wfc_sb[:, s0:s1, :], wfc_r[:, s0:s1, :])
            wout_sb = sb.tile([P, IC, H], FP)
            wout_r = wout.rearrange("(ic p) c -> p ic c", p=P)
            nck2 = min(4, IC)
            for cc in range(nck2):
                s0, s1 = cc * IC // nck2, (cc + 1) * IC // nck2
                nc.sync.dma_start(wout_sb[:, s0:s1, :], wout_r[:, s0:s1, :])

            xT_sb = sb.tile([P, HC * B], FP)
            for hcc in range(HC):
                pt = ps.tile([P, B], FP, tag="pt")
                nc.tensor.transpose(pt[:], xh_sb[:, hcc * P:(hcc + 1) * P], idB_sb[:])
                nc.scalar.copy(xT_sb[:, hcc * B:(hcc + 1) * B], pt[:])

            # fc: x-stationary, W moving -> psum_u [B, I]
            psum_u = psu.tile([B, I], FP)
            for nn in range(I // NW):
                for hcc in range(HC):
                    nc.tensor.matmul(
                        psum_u[:, nn * NW:(nn + 1) * NW],
                        xT_sb[:, hcc * B:(hcc + 1) * B],
                        wfc_sb[:, hcc, nn * NW:(nn + 1) * NW],
                        start=(hcc == 0), stop=(hcc == HC - 1),
                    )
            u_sb = sb.tile([B, I], FP)
            nc.vector.tensor_copy(u_sb[:], psum_u[:])

            # transpose u -> uT chunks, gelu in transposed domain
            g_sb = sb.tile([P, IC * B], FP)
            c_gelu = float(np.sqrt(2.0 / np.pi))
            for ic in range(IC):
                pt2 = ps.tile([P, B], FP, tag="pt")
                nc.tensor.transpose(pt2[:], u_sb[:, ic * P:(ic + 1) * P], idB_sb[:])
                # u = uT + bias; gelu_new(u) = 0.5 u (1 + tanh(c (u + 0.044715 u^3)))
                u = sb.tile([P, B], FP, tag="u")
                nc.scalar.activation(u[:], pt2[:], AF.Identity, bias=bfc_sb[:, ic:ic + 1])
                t = sb.tile([P, B], FP, tag="t")
                nc.vector.tensor_mul(t[:], u[:], u[:])
                nc.vector.tensor_mul(t[:], t[:], u[:])
                nc.vector.tensor_scalar_mul(t[:], t[:], 0.044715)
                nc.vector.tensor_add(t[:], t[:], u[:])
                nc.scalar.activation(t[:], t[:], AF.Tanh, scale=c_gelu)
                nc.vector.tensor_scalar_add(t[:], t[:], 1.0)
                nc.vector.tensor_mul(t[:], t[:], u[:])
                nc.vector.tensor_scalar_mul(
                    g_sb[:, ic * B:(ic + 1) * B], t[:], 0.5
                )

            # out proj: g-stationary, W_out moving -> psum_y [B, H]
            psum_y = psu.tile([B, H], FP)
            for nn in range(H // NWH):
                for ic in range(IC):
                    nc.tensor.matmul(
                        psum_y[:, nn * NWH:(nn + 1) * NWH],
                        g_sb[:, ic * B:(ic + 1) * B],
                        wout_sb[:, ic, nn * NWH:(nn + 1) * NWH],
                        start=(ic == 0), stop=(ic == IC - 1),
                    )
            y_sb = sb.tile([B, H], FP)
            nc.vector.tensor_copy(y_sb[:], psum_y[:])
            nc.sync.dma_start(ypart[:], y_sb[:])
    return nc


# ---------------------------------------------------------------------------
# Host orchestration
# ---------------------------------------------------------------------------
def _phase1_inmaps(hidden, cached_k, cached_v, ln1_g, ln1_b, W_qkv, b_qkv, W_proj,
                   M=8, NHL=2, HD=128):
    B, H = hidden.shape
    s = 1.0 / np.sqrt(HD)
    ident = np.eye(128, dtype=np.float32)
    identB = np.eye(B, dtype=np.float32)
    onesc = np.ones((128, 1), np.float32)
    onesr = np.ones((1, 128), np.float32)
    g_bc = np.ascontiguousarray(np.broadcast_to(ln1_g, (B, H)), np.float32)
    b_bc = np.ascontiguousarray(np.broadcast_to(ln1_b, (B, H)), np.float32)
    maps = []
    for c in range(M):
        lo, hi = c * NHL * HD, (c + 1) * NHL * HD
        wq = W_qkv[:, lo:hi]
        wk = W_qkv[:, H + lo:H + hi]
        wv = W_qkv[:, 2 * H + lo:2 * H + hi]
        wqkv_c = np.ascontiguousarray(np.concatenate([wq, wk, wv], axis=1), np.float32)
        bq = b_qkv[lo:hi] * s          # pre-scale q bias
        bk = b_qkv[H + lo:H + hi]
        bv = b_qkv[2 * H + lo:2 * H + hi]
        bqkv_c = np.ascontiguousarray(np.concatenate([bq, bk, bv]), np.float32)
        maps.append({
            "hid": hidden,
            "ln1g": g_bc,
            "ln1b": b_bc,
            "wqkv": wqkv_c,
            "bqkv": bqkv_c,
            "kc": np.ascontiguousarray(cached_k[:, c * NHL:(c + 1) * NHL], np.float32),
            "vc": np.ascontiguousarray(cached_v[:, c * NHL:(c + 1) * NHL], np.float32),
            "wproj": np.ascontiguousarray(W_proj[lo:hi, :], np.float32),
            "ident": ident,
            "identB": identB,
            "onesc": onesc,
            "onesr": onesr,
        })
    return maps


def _phase2_inmaps(xh2, W_fc, b_fc, W_out, M=8):
    B, H = xh2.shape
    I = W_fc.shape[1] // M
    identB = np.eye(B, dtype=np.float32)
    maps = []
    for c in range(M):
        maps.append({
            "xh2": xh2,
            "wfc": np.ascontiguousarray(W_fc[:, c * I:(c + 1) * I], np.float32),
            "bfc": np.ascontiguousarray(b_fc[c * I:(c + 1) * I], np.float32),
            "wout": np.ascontiguousarray(W_out[c * I:(c + 1) * I, :], np.float32),
            "identB": identB,
        })
    return maps


_CACHE = {}


def _get_programs():
    if "nc1" not in _CACHE:
        nc1 = build_phase1(nc_factory=_hw_nc)
        nc1.compile()
        nc2 = build_phase2(nc_factory=_hw_nc)
        nc2.compile()
        _CACHE["nc1"] = nc1
        _CACHE["nc2"] = nc2
    return _CACHE["nc1"], _CACHE["nc2"]


def kernel(hidden_states, cached_k, cached_v, ln1_g, ln1_b, W_qkv, b_qkv,
           W_proj, b_proj, ln2_g, ln2_b, W_fc, b_fc, W_out, b_out,
           _trace=False, _timings=None):
    M = 8
    B, _, H = hidden_states.shape
    hid = np.ascontiguousarray(hidden_states[:, 0, :], np.float32)

    nc1, nc2 = _get_programs()

    maps1 = _phase1_inmaps(hid, cached_k, cached_v, ln1_g, ln1_b,
                           W_qkv, b_qkv, W_proj, M=M)
    r1 = run_bass_kernel_spmd(nc1, maps1, list(range(M)), trace=_trace)
    if _timings is not None:
        _timings.append(r1.exec_time_ns)
    hparts = [r1.results[i]["hpart"] for i in range(M)]
    h = np.sum(hparts, axis=0) + np.asarray(b_proj) + hid

    mu = h.mean(-1, keepdims=True)
    var = ((h - mu) ** 2).mean(-1, keepdims=True)
    xh2 = ((h - mu) / np.sqrt(var + EPS) * np.asarray(ln2_g)
           + np.asarray(ln2_b)).astype(np.float32)

    maps2 = _phase2_inmaps(xh2, W_fc, b_fc, W_out, M=M)
    r2 = run_bass_kernel_spmd(nc2, maps2, list(range(M)), trace=_trace)
    if _timings is not None:
        _timings.append(r2.exec_time_ns)
    yparts = [r2.results[i]["ypart"] for i in range(M)]
    y = np.sum(yparts, axis=0) + np.asarray(b_out) + h
    return y[:, None, :].astype(np.float32)

